# revision 12
# baseline (speedup 1.0000x reference)
"""Distributed Trainium2 kernel for nn_Attention_49529562858354.

Reference computation (per batch): LayerNorm(x) @ w_qkv -> 16-head
self-attention with key-side masking (mask==1 -> key excluded).

Sharding (8 cores): core = batch * 2 + head_group. Data parallel over
the 4 batches, tensor parallel over 2 groups of 8 heads. Each core gets
its batch's x, the w_qkv column slice for its heads, and produces
out[:, hg*512:(hg+1)*512] for its batch. No collectives needed.

Host-side prep: masked keys removed (gather) and padded to a multiple
of 128 with gate=0 rows (roughly halves attention work); ln_g folded
into w_qkv columns; x/xkv/weights cast to bf16 (halves input DMA and
doubles DVE rate on the LayerNorm path).

Per-core pipeline (bf16 compute, f32 LN stats):
  1. LayerNorm in natural layout (bn_stats; rstd = exp(-0.5*ln(var+eps))
     batched per chunk -- Ln/Exp share one ACT table set with the
     attention exp). kv rows fold in the key gate (pad keys -> zero
     rows -> zero V rows and denominator entries). xhat (bf16) is
     PE-transposed tile-by-tile into PSUM and copied to xkvT/xqT in
     SBUF -- no DRAM scratch round trip, PE stays busy (HAM warm).
  2. QKV projections: qT/kT as [cols, tokens]; v natural with a gate
     column per head (softmax denominator comes out of the AV matmul).
  3. Attention: scores transposed [k, q] in PSUM, exp on ScalarE with
     fused scale (no max subtraction; post-LN logits are O(1)), AV
     accumulates [65, 512] with row 64 = denominator. Epilogue: PE
     transpose, reciprocal, scale into [128, 512] out tiles.
     Later q chunks prefetch their LayerNorm + qT mid-attention.

Engine/queue map: x loads -> gpsimd (SWDGE); weight loads + outputs ->
sync; ScalarE runs only activations (exp is the bottleneck engine).
"""

import os
import sys
import types

for _p in ("/opt/trn_rl_repo", "/root/.axon_site"):
    if _p not in sys.path:
        sys.path.insert(0, _p)

import numpy as np
import ml_dtypes

import concourse.bass as bass
import concourse.tile as tile
from concourse import mybir

N_CORES = 8
N_TOK = 2048
DIM = 1024
HEADS_LOCAL = 8
DH = 64
COLS = HEADS_LOCAL * DH
SCALE = DH ** -0.5
EPS = 1e-5
QCHUNK = 512
KGROUP = 3
FP8AV = os.environ.get("KERNEL_FP8AV", "1") == "1"
EXPB = -1.0  # exp bias keeps unnormalized probs in fp8e4 range; cancels in softmax

F32 = mybir.dt.float32
BF16 = mybir.dt.bfloat16
F8 = mybir.dt.float8e4
MUL = mybir.AluOpType.mult
ADD = mybir.AluOpType.add

LAST_EXEC_TIME_NS = None


def _split_excess_waits(nc, max_waits=1, max_updates=1):
    """This container's walrus rejects >1 sync wait/update per
    instruction; move overflow onto adjacent same-engine NoOps."""
    counter = [0]

    def fresh():
        counter[0] += 1
        return f"I-WFIX-{counter[0]}"

    for f in nc.m.functions:
        for blk in f.blocks:
            il = blk.instructions
            out = []
            changed = False
            for inst in il:
                si = inst.sync_info
                if si is None:
                    out.append(inst)
                    continue
                waits = list(si.on_wait or [])
                updates = list(si.on_update or [])
                pre, post = [], []
                if len(waits) > max_waits:
                    for w in waits[max_waits:]:
                        nop = mybir.InstNoOp(name=fresh(), ins=[], outs=[])
                        nop.engine = inst.engine
                        nop.sync_info = mybir.SyncInfo(on_wait=[w], on_update=[])
                        pre.append(nop)
                    waits = waits[:max_waits]
                if len(updates) > max_updates:
                    for u in updates[max_updates:]:
                        nop = mybir.InstNoOp(name=fresh(), ins=[], outs=[])
                        nop.engine = inst.engine
                        nop.sync_info = mybir.SyncInfo(on_wait=[], on_update=[u])
                        post.append(nop)
                    updates = updates[:max_updates]
                if pre or post:
                    inst.sync_info = mybir.SyncInfo(on_wait=waits, on_update=updates)
                    changed = True
                out.extend(pre)
                out.append(inst)
                out.extend(post)
            if changed:
                blk.instructions = out


def build_graph(l_kv, has_bias):
    lt = l_kv // 128
    nc = bass.Bass()

    x_ext = nc.declare_dram_parameter("x", [N_TOK, DIM], BF16, isOutput=False)
    xkv_ext = nc.declare_dram_parameter("xkv", [l_kv, DIM], BF16, isOutput=False)
    gate_ext = nc.declare_dram_parameter("gate", [l_kv], F32, isOutput=False)
    gate_rep_ext = nc.declare_dram_parameter(
        "gate_rep", [128, lt * HEADS_LOCAL], F32, isOutput=False
    )
    wq_ext = nc.declare_dram_parameter("wq", [DIM, COLS], BF16, isOutput=False)
    wk_ext = nc.declare_dram_parameter("wk", [DIM, COLS], BF16, isOutput=False)
    wv_ext = nc.declare_dram_parameter("wv", [DIM, COLS], BF16, isOutput=False)
    b_ext = nc.declare_dram_parameter("ln_b", [DIM], F32, isOutput=False)
    out_ext = nc.declare_dram_parameter("out", [N_TOK, COLS], F32, isOutput=True)

    NQCH = N_TOK // QCHUNK
    # 512-token-aligned kv chunks (LN rstd batching granularity).
    kv_chunks = []
    off = 0
    while off < l_kv:
        nr = min(512, l_kv - off)
        kv_chunks.append((off, nr))
        off += nr

    with tile.TileContext(nc) as tc:
        import contextlib

        with contextlib.ExitStack() as ctx:
            singles = ctx.enter_context(tc.tile_pool(name="singles", bufs=1))
            xin = ctx.enter_context(tc.tile_pool(name="xin", bufs=8))
            stats = ctx.enter_context(tc.tile_pool(name="stats", bufs=4))
            xhat_pool = ctx.enter_context(tc.tile_pool(name="xhat", bufs=4))
            p_pool = ctx.enter_context(tc.tile_pool(name="p_sb", bufs=2))
            o_pool = ctx.enter_context(tc.tile_pool(name="o_sb", bufs=2))
            out_pool = ctx.enter_context(tc.tile_pool(name="outt", bufs=2))
            recip_pool = ctx.enter_context(tc.tile_pool(name="recip", bufs=2))
            # PSUM (8 banks): s0/s1 = score buffers (3 banks each, used
            # alternately -> double buffering), "o" = AV accumulator +
            # epilogue transposes (1 bank), "proj" = projection chains +
            # xhat transposes (1 bank).
            psum = ctx.enter_context(tc.tile_pool(name="psum", bufs=1, space="PSUM"))

            # --- constants -------------------------------------------------
            gate_sb = singles.tile([128, lt], F32, tag="gate_sb")
            nc.sync.dma_start(
                out=gate_sb[:], in_=gate_ext.rearrange("(t p) -> p t", p=128)
            )
            gate_rep_sb = singles.tile([128, lt * HEADS_LOCAL], F32, tag="gate_rep_sb")
            nc.sync.dma_start(out=gate_rep_sb[:], in_=gate_rep_ext[:, :])
            if has_bias:
                b_sb = singles.tile([128, 8], F32, tag="b_sb")
                nc.sync.dma_start(
                    out=b_sb[:], in_=b_ext.rearrange("(kd p) -> p kd", p=128)
                )
            eps_sb = singles.tile([128, 1], F32, tag="eps_sb")
            nc.vector.memset(eps_sb[:], EPS)
            ident = singles.tile([128, 128], BF16, tag="ident")
            from concourse.masks import make_identity

            make_identity(nc, ident[:])

            # Warm up the PE (HAM clock gate) on the identity -- available
            # within ~1us, long before the first weight DMA lands.
            wu = psum.tile([128, 512], F32, tag="proj", name="warmup_ps")
            for i in range(32):
                nc.tensor.matmul(
                    wu[:, 0:128], ident[:], ident[:], start=True, stop=True
                )

            # --- weights: bf16, ln_g already folded on host ---------------
            wg = {}
            for name, ext in (("v", wv_ext), ("k", wk_ext), ("q", wq_ext)):
                tiles = []
                for kd in range(8):
                    wb = singles.tile(
                        [128, COLS], BF16, tag=f"wg_{name}_{kd}", name=f"wg_{name}{kd}"
                    )
                    nc.sync.dma_start(
                        out=wb[:], in_=ext[kd * 128 : (kd + 1) * 128, :]
                    )
                    tiles.append(wb)
                wg[name] = tiles

            # --- transposed activations (d on partitions) ------------------
            xkvT = singles.tile([128, 8 * l_kv], BF16, tag="xkvT")
            xqT = singles.tile([128, 8 * N_TOK], BF16, tag="xqT")
            xkvT_r = xkvT.rearrange("p (k t) -> p k t", k=8)
            xqT_r = xqT.rearrange("p (k t) -> p k t", k=8)

            # PSUM tag rotation: full rotation before attention starts,
            # "proj" only after.
            PROJ_TAGS = ("proj", "o", "s0", "s1")
            proj_n = [0]
            attn_started = [False]

            def proj_psum(n_free, name, dtype=F32):
                if attn_started[0]:
                    return psum.tile([128, n_free], dtype, tag="proj", name=name)
                tag = PROJ_TAGS[proj_n[0] % 4]
                proj_n[0] += 1
                return psum.tile([128, n_free], dtype, tag=tag, name=name)

            # --- LayerNorm + PE transpose for one row chunk ---------------
            def prep_chunk(src_ext, xT_r, row0, nrows, gated, pfx):
                nt = nrows // 128
                tb0 = row0 // 128
                xts = []
                mva = stats.tile([128, 2, nt], F32, tag="mva", name=f"mva_{pfx}{tb0}")
                for t in range(nt):
                    xt = xin.tile([128, DIM], BF16, tag="xin", name=f"x_{pfx}{tb0+t}")
                    nc.gpsimd.dma_start(
                        out=xt[:],
                        in_=src_ext[row0 + t * 128 : row0 + (t + 1) * 128, :],
                    )
                    st = stats.tile(
                        [128, 2, 6], F32, tag="bnst", name=f"st_{pfx}{tb0+t}"
                    )
                    xgr = xt.rearrange("p (s d) -> p s d", s=2)
                    nc.vector.bn_stats(out=st[:, 0, :], in_=xgr[:, 0, :])
                    nc.vector.bn_stats(out=st[:, 1, :], in_=xgr[:, 1, :])
                    nc.vector.bn_aggr(out=mva[:, :, t], in_=st[:])
                    xts.append(xt)
                lv = stats.tile([128, nt], F32, tag="lv", name=f"lv_{pfx}{tb0}")
                nc.scalar.activation(
                    out=lv[:],
                    in_=mva[:, 1, :],
                    func=mybir.ActivationFunctionType.Ln,
                    bias=eps_sb[:],
                    scale=1.0,
                )
                rstd = stats.tile([128, nt], F32, tag="rstd", name=f"rs_{pfx}{tb0}")
                nc.scalar.activation(
                    out=rstd[:],
                    in_=lv[:],
                    func=mybir.ActivationFunctionType.Exp,
                    scale=-0.5,
                )
                if gated:
                    nc.vector.tensor_mul(
                        rstd[:], rstd[:], gate_sb[:, tb0 : tb0 + nt]
                    )
                for t in range(nt):
                    nmr = stats.tile([128, 1], F32, tag="nmr", name=f"nm_{pfx}{tb0+t}")
                    nc.vector.tensor_scalar(
                        out=nmr[:], in0=mva[:, 0, t : t + 1],
                        scalar1=rstd[:, t : t + 1], scalar2=-1.0, op0=MUL, op1=MUL,
                    )
                    xh = xhat_pool.tile(
                        [128, DIM], BF16, tag="xhat", name=f"xh_{pfx}{tb0+t}"
                    )
                    nc.vector.tensor_scalar(
                        out=xh[:], in0=xts[t][:], scalar1=rstd[:, t : t + 1],
                        scalar2=nmr[:], op0=MUL, op1=ADD,
                    )
                    psT = proj_psum(DIM, f"psT_{pfx}{tb0+t}", dtype=BF16)
                    psT_r = psT.rearrange("p (k t) -> p k t", k=8)
                    for kd in range(8):
                        nc.tensor.transpose(
                            psT_r[:, kd, :],
                            xh[:, kd * 128 : (kd + 1) * 128],
                            ident[:],
                        )
                    r0 = row0 + t * 128
                    nc.vector.tensor_copy(
                        xT_r[:, :, r0 : r0 + 128], psT_r[:]
                    )
                if has_bias:
                    for kd in range(8):
                        nc.vector.tensor_scalar(
                            out=xT_r[:, kd, row0 : row0 + nrows],
                            in0=xT_r[:, kd, row0 : row0 + nrows],
                            scalar1=b_sb[:, kd : kd + 1],
                            scalar2=None,
                            op0=ADD,
                        )

            # --- v projection + vaug (gate already folded into xhat_kv) ---
            # With FP8AV, kv tiles (j, KGROUP+j) pair up: their V columns go
            # interleaved (fp8) into va_dr so one DoubleRow matmul contracts
            # both tiles; p arrives fp8-interleaved from the exp. Leftover
            # tiles (>= 2*KGROUP) keep the bf16 path. 66-wide head slots keep
            # the DoubleRow weight step a multiple of 16 bytes.
            ngroups = (lt + KGROUP - 1) // KGROUP
            paired = FP8AV and lt >= 2 * KGROUP
            vaug = {}
            va_dr = {}

            def v_proj(tb):
                ps = proj_psum(COLS, f"psv{tb}")
                for kd in range(8):
                    nc.tensor.matmul(
                        ps[:],
                        xkvT_r[:, kd, tb * 128 : (tb + 1) * 128],
                        wg["v"][kd][:],
                        start=(kd == 0),
                        stop=(kd == 7),
                    )
                if paired and tb < 2 * KGROUP:
                    pj, o = tb % KGROUP, tb // KGROUP
                    if o == 0:
                        va_dr[pj] = singles.tile(
                            [128, 2 * HEADS_LOCAL * 66], F8,
                            tag=f"vadr_{pj}", name=f"vadr{pj}",
                        )
                    vr = va_dr[pj].rearrange("p (o h c) -> p o h c", o=2, c=66)
                    nc.vector.tensor_copy(
                        vr[:, o, :, 0:64], ps.rearrange("p (h c) -> p h c", c=64)
                    )
                    nc.vector.tensor_copy(
                        vr[:, o, :, 64],
                        gate_rep_sb[:, tb * HEADS_LOCAL : (tb + 1) * HEADS_LOCAL],
                    )
                    return
                va = singles.tile(
                    [128, HEADS_LOCAL * 65], BF16, tag=f"vaug_{tb}", name=f"vaug{tb}"
                )
                va_r = va.rearrange("p (h c) -> p h c", c=65)
                nc.vector.tensor_copy(
                    va_r[:, :, 0:64], ps.rearrange("p (h c) -> p h c", c=64)
                )
                nc.vector.tensor_copy(
                    va_r[:, :, 64],
                    gate_rep_sb[:, tb * HEADS_LOCAL : (tb + 1) * HEADS_LOCAL],
                )
                vaug[tb] = va

            # Interleave kv LN/transpose with v projections so the PE FIFO
            # never waits on a later chunk's LayerNorm.
            tb_done = 0
            for c, (row0, nrows) in enumerate(kv_chunks):
                prep_chunk(xkv_ext, xkvT_r, row0, nrows, True, "kv")
                for tb in range(tb_done, (row0 + nrows) // 128):
                    v_proj(tb)
                tb_done = (row0 + nrows) // 128

            prep_chunk(x_ext, xqT_r, 0, QCHUNK, False, "q")

            # --- kT/qT projections + attention -----------------------------
            kproj_chunks = list(kv_chunks)
            kT = [
                singles.tile([128, l_kv], BF16, tag=f"kT_{cb}", name=f"kT{cb}")
                for cb in range(4)
            ]
            qT = [
                singles.tile([128, N_TOK], BF16, tag=f"qT_{cb}", name=f"qT{cb}")
                for cb in range(4)
            ]
            ngroups = (lt + KGROUP - 1) // KGROUP

            def kT_proj(cb):
                for row0, nrows in kproj_chunks:
                    ps = proj_psum(512, f"psk{cb}_{row0}")
                    for kd in range(8):
                        nc.tensor.matmul(
                            ps[:, :nrows],
                            wg["k"][kd][:, cb * 128 : (cb + 1) * 128],
                            xkvT_r[:, kd, row0 : row0 + nrows],
                            start=(kd == 0),
                            stop=(kd == 7),
                        )
                    nc.vector.tensor_copy(
                        kT[cb][:, row0 : row0 + nrows], ps[:, :nrows]
                    )

            def qT_proj(tcn, cbs=range(4)):
                for cb in cbs:
                    ps = proj_psum(512, f"psq{cb}_{tcn}")
                    for kd in range(8):
                        nc.tensor.matmul(
                            ps[:],
                            wg["q"][kd][:, cb * 128 : (cb + 1) * 128],
                            xqT_r[:, kd, tcn * 512 : (tcn + 1) * 512],
                            start=(kd == 0),
                            stop=(kd == 7),
                        )
                    nc.vector.tensor_copy(qT[cb][:, tcn * 512 : (tcn + 1) * 512], ps[:])

            sidx = [0]

            def emit_qk_alloc(qc, h, gi):
                i = sidx[0]
                sidx[0] ^= 1
                gsz = min(KGROUP, lt - gi * KGROUP)
                ps_s = psum.tile(
                    [128, KGROUP * 512], F32, tag=f"s{i}", name=f"ps{qc}_{h}_{gi}"
                )
                return [qc, h, gi, ps_s, i, gsz]

            def emit_qk_mm(meta, k):
                qc, h, gi, ps_s, i, gsz = meta
                if k >= gsz:
                    return
                cb = h // 2
                p0 = (h % 2) * 64
                tb = gi * KGROUP + k
                nc.tensor.matmul(
                    ps_s[:, k * 512 : (k + 1) * 512],
                    kT[cb][p0 : p0 + 64, tb * 128 : (tb + 1) * 128],
                    qT[cb][p0 : p0 + 64, qc * 512 : (qc + 1) * 512],
                    start=True,
                    stop=True,
                )

            def emit_exp(qc, h, gi, qk):
                ps_s, i, gsz = qk[3], qk[4], qk[5]
                p_sb = p_pool.tile(
                    [128, KGROUP * 512], BF16, tag=f"p{i}", name=f"p{qc}_{h}_{gi}"
                )
                nc.scalar.activation(
                    out=p_sb[:, : gsz * 512],
                    in_=ps_s[:, : gsz * 512],
                    func=mybir.ActivationFunctionType.Exp,
                    scale=SCALE,
                )
                return p_sb

            def emit_av(qc, h, gi, gsz, p_sb, po):
                for k in range(gsz):
                    tb = gi * KGROUP + k
                    nc.tensor.matmul(
                        po[:],
                        vaug[tb][:, h * 65 : (h + 1) * 65],
                        p_sb[:, k * 512 : (k + 1) * 512],
                        start=(tb == 0),
                        stop=(tb == lt - 1),
                    )

            def emit_epilogue(qc, h, po, out_tiles):
                o_sb = o_pool.tile([65, 512], BF16, tag="o_sb", name=f"ob{qc}_{h}")
                nc.vector.tensor_copy(o_sb[:], po[:])
                # 66-wide slots keep bf16 PSUM writes 4-byte aligned.
                pt = psum.tile([128, 4 * 66], BF16, tag="o", name=f"pt{qc}_{h}")
                for j in range(4):
                    nc.tensor.transpose(
                        pt[:, j * 66 : (j + 1) * 66],
                        o_sb[:, j * 128 : (j + 1) * 128],
                        ident[0:65, 0:66],
                    )
                rc = recip_pool.tile([128, 4], F32, tag="recip", name=f"rc{qc}_{h}")
                nc.vector.reciprocal(
                    out=rc[:],
                    in_=pt.rearrange("p (j c) -> p j c", c=66)[:, :, 64:65],
                )
                for j in range(4):
                    nc.vector.tensor_scalar(
                        out=out_tiles[j][:, h * 64 : (h + 1) * 64],
                        in0=pt[:, j * 66 : j * 66 + 64],
                        scalar1=rc[:, j : j + 1],
                        scalar2=None,
                        op0=MUL,
                    )

            def attention_stretch(groups, out_tiles, mid_cb=None):
                """Software-pipelined: QK of group n+1 is emitted before the
                exp/AV of group n so the PE FIFO never head-of-line blocks
                the next score matmuls behind an exp-waiting AV.
                groups: list of (qc, h, gi). mid_cb: callback emitted after
                the tail of groups[mid_idx] (for mid-stretch prefetch)."""
                po_map = {}
                qks = {}
                pending_epi = [None]

                def flush_epi():
                    if pending_epi[0] is not None:
                        emit_epilogue(*pending_epi[0])
                        pending_epi[0] = None

                # Emission units: a cross-head group transition emits both
                # groups' QK matmuls interleaved ktile-by-ktile, so adjacent
                # matmuls hit different PE row groups and run concurrently.
                units = []
                i = 0
                while i < len(groups):
                    if i + 1 < len(groups) and groups[i][1] != groups[i + 1][1]:
                        units.append((i, i + 1))
                        i += 2
                    else:
                        units.append((i,))
                        i += 1
                unit_of = {}
                for ui, u in enumerate(units):
                    for idx in u:
                        unit_of[idx] = ui
                emitted = set()

                def emit_unit(ui):
                    if ui in emitted:
                        return
                    emitted.add(ui)
                    metas = [emit_qk_alloc(*groups[idx]) for idx in units[ui]]
                    for k in range(KGROUP):
                        for m in metas:
                            emit_qk_mm(m, k)
                    for j, idx in enumerate(units[ui]):
                        qks[idx] = metas[j]

                emit_unit(0)
                for idx, (qc, h, gi) in enumerate(groups):
                    p_sb = emit_exp(qc, h, gi, qks[idx])
                    gsz = qks.pop(idx)[5]
                    if idx + 1 < len(groups):
                        emit_unit(unit_of[idx + 1])
                    flush_epi()
                    if gi == 0:
                        po_map[h] = psum.tile(
                            [65, 512], F32, tag="o", name=f"po{qc}_{h}"
                        )
                    emit_av(qc, h, gi, gsz, p_sb, po_map[h])
                    if gi == ngroups - 1:
                        pending_epi[0] = (qc, h, po_map.pop(h), out_tiles)
                    if mid_cb is not None and idx == len(groups) // 2:
                        mid_cb()
                        mid_cb = None
                flush_epi()

            def make_out_tiles(qc):
                return [
                    out_pool.tile([128, COLS], F32, tag=f"out_{j}", name=f"o{qc}_{j}")
                    for j in range(4)
                ]

            # qc0: attention head pairs interleaved with kT/qT projections;
            # the next column block's projections are emitted mid-stretch so
            # stretch boundaries never wait on them.
            out_tiles = make_out_tiles(0)
            kT_proj(0)
            qT_proj(0, cbs=[0])

            def mk_cb_prefetch(cbn):
                def cbk():
                    if cbn < 4:
                        kT_proj(cbn)
                        qT_proj(0, cbs=[cbn])
                    else:
                        prep_chunk(x_ext, xqT_r, QCHUNK, QCHUNK, False, "q")
                        qT_proj(1)
                return cbk

            attn_started[0] = True
            for cb in range(4):
                attention_stretch(
                    [(0, h, gi) for h in (2 * cb, 2 * cb + 1) for gi in range(ngroups)],
                    out_tiles,
                    mid_cb=mk_cb_prefetch(cb + 1),
                )
            for j in range(4):
                nc.sync.dma_start(
                    out=out_ext[j * 128 : (j + 1) * 128, :], in_=out_tiles[j][:]
                )

            for qc in range(1, NQCH):
                out_tiles = make_out_tiles(qc)

                def mk_prefetch(qc):
                    def cb():
                        if qc + 1 < NQCH:
                            prep_chunk(
                                x_ext, xqT_r, (qc + 1) * QCHUNK, QCHUNK, False, "q"
                            )
                            qT_proj(qc + 1)
                    return cb

                attention_stretch(
                    [(qc, h, gi) for h in range(HEADS_LOCAL) for gi in range(ngroups)],
                    out_tiles,
                    mid_cb=mk_prefetch(qc),
                )
                for j in range(4):
                    row0 = qc * QCHUNK + j * 128
                    nc.sync.dma_start(
                        out=out_ext[row0 : row0 + 128, :], in_=out_tiles[j][:]
                    )

    _split_excess_waits(nc)
    return nc


_GRAPH_CACHE = {}


def kernel(x, mask, w_qkv, ln_g, ln_b):
    x = np.asarray(x, dtype=np.float32)
    mask = np.asarray(mask)
    w_qkv = np.asarray(w_qkv, dtype=np.float32)
    ln_g = np.asarray(ln_g, dtype=np.float32)
    ln_b = np.asarray(ln_b, dtype=np.float32)
    b, n, d = x.shape

    keeps = [np.where(mask[bi] == 0)[0] for bi in range(b)]
    l_kv = max(128, -(-max(len(k) for k in keeps) // 128) * 128)
    lt = l_kv // 128
    has_bias = bool(np.any(ln_b != 0.0))

    global LAST_EXEC_TIME_NS
    key = (l_kv, has_bias)
    if key not in _GRAPH_CACHE:
        _GRAPH_CACHE[key] = build_graph(l_kv, has_bias)
    nc = _GRAPH_CACHE[key]

    # ln_g folds into the weight columns (exact f32 multiply on host).
    wg_full = (w_qkv * ln_g[:, None]).astype(ml_dtypes.bfloat16)
    x_bf = x.astype(ml_dtypes.bfloat16)

    in_maps = []
    for core in range(N_CORES):
        bi, hg = core // 2, core % 2
        keep = keeps[bi]
        xkv = np.zeros((l_kv, d), dtype=ml_dtypes.bfloat16)
        xkv[: len(keep)] = x_bf[bi][keep]
        gate = np.zeros((l_kv,), dtype=np.float32)
        gate[: len(keep)] = 1.0
        gate_rep = np.repeat(
            gate.reshape(lt, 128).T[:, :, None], HEADS_LOCAL, axis=2
        ).reshape(128, lt * HEADS_LOCAL)
        m = {
            "x": x_bf[bi],
            "xkv": xkv,
            "gate": gate,
            "gate_rep": np.ascontiguousarray(gate_rep),
            "wq": np.ascontiguousarray(wg_full[:, hg * COLS : (hg + 1) * COLS]),
            "wk": np.ascontiguousarray(
                wg_full[:, d + hg * COLS : d + (hg + 1) * COLS]
            ),
            "wv": np.ascontiguousarray(
                wg_full[:, 2 * d + hg * COLS : 2 * d + (hg + 1) * COLS]
            ),
            "ln_b": ln_b,
        }
        in_maps.append(m)

    from concourse.bass_utils import run_bass_kernel_spmd

    trace = os.environ.get("KERNEL_TRACE", "") == "1"
    kwargs = {}
    if trace:
        import antenv

        if "antenv.axon_hooks" not in sys.modules:
            hooks = types.ModuleType("antenv.axon_hooks")
            hooks._hook = None
            hooks.set_axon_ntff_profile_hook = lambda h: setattr(hooks, "_hook", h)
            hooks.get_axon_ntff_profile_hook = lambda: hooks._hook
            sys.modules["antenv.axon_hooks"] = hooks
            antenv.axon_hooks = hooks
        from trn_agent_boot.trn_boot import _ntff_profile_via_ctypes

        sys.modules["antenv.axon_hooks"].set_axon_ntff_profile_hook(
            _ntff_profile_via_ctypes("/opt/axon/libaxon_pjrt.so")
        )
        from concourse import bass_utils

        bass_utils.upload_artifacts = lambda tmpdir: tmpdir
        import uuid

        tdir = os.path.join(
            os.environ.get("KERNEL_TRACE_DIR", "/tmp/kernel_trace"),
            uuid.uuid4().hex[:8],
        )
        os.makedirs(tdir, exist_ok=True)
        kwargs = {"trace": True, "tmpdir": tdir}

    res = run_bass_kernel_spmd(nc, in_maps, core_ids=list(range(N_CORES)), **kwargs)
    LAST_EXEC_TIME_NS = res.exec_time_ns

    out = np.empty((b, n, d), dtype=np.float32)
    for core in range(N_CORES):
        bi, hg = core // 2, core % 2
        out[bi][:, hg * COLS : (hg + 1) * COLS] = res.results[core]["out"]
    return out
